# revision 1
# baseline (speedup 1.0000x reference)
"""Trainium2 Bass kernel for nn_MinibatchDiscrimination (B=256, F=1024, O=128, K=8).

Computes out = concat([x, c], axis=1) where
    M = (x @ T.reshape(F, O*K)).reshape(B, O, K)
    c[i, o] = sum_{j != i} exp(-sum_k |M[j,o,k] - M[i,o,k]|)
(the reference's `sum_j exp(-d) - 1` equals the self-term-excluded sum since
 d_ii == 0 exactly).

Distribution: batch rows of c are sharded across 8 cores (32 rows each).
Every core redundantly computes the full GEMM (it needs all of M for the
pairwise reduction anyway; the GEMM is ~7% of the work).  SPMD divergence is
achieved purely through data: core b receives x^T with its batch columns
rotated so that its 32 owned rows sit at local columns [0, 32) — the pairwise
sum over j is permutation-invariant, so compile-time index offsets work for
every core.

Per-core device pipeline (all layouts use partition p = (o%64)*2 + (k%2),
column group g = 4*(o//64) + ((k//2)%4), i.e. 2 o-halves x 4 k-quarters;
matmul outputs may only be placed at partition offsets {0, 64}):
  1. DMA  w, xt (fp8e4m3, pre-interleaved SBUF images), masks (bf16).
  2. GEMM Mt[p, g*B + j] = M[j, o(p,g), k(p,g)]   (TensorE, fp8, f32 PSUM),
     copied to SBUF as bf16 (full) + f32 (first 32 columns, tensor_scalar
     operands must be f32).
  2b. S[o, j] = sum_k M[j,o,k] via a mask-matmul over Mt; stored as
     -S/2 (f32, per (o,j)) and -S[:, :32] (f32 bias columns).
  3. For each local row i in [0,32), using |d| = 2*relu(d) - d and
     sum_k d_k = S_j - S_i:
     a. VectorE tensor_scalar:  R = relu(Mt - Mt[:, i])  (fused
        subtract+max-0, bf16, 4x mode), 8 column groups.
     b. ScalarE prefills the PSUM tile with -S_j/2, then TensorE mask-matmuls
        accumulate (start=False): h[o, j] = sum_k relu(d_k) - S[o,j]/2.
     c. ScalarE activation Exp(scale=-2, bias=-S[:, i]) with accum_out ->
        c_acc[:, i] = sum_j exp(-2h - S_i) = sum_j exp(-diffs);
        the self term e[:, i] is extracted to c_diag[:, i] (this also keeps
        the diagonal cancellation exact).
  4. c = c_acc - c_diag (VectorE), DMA out as (O, 32) f32.
Host gathers: c_full rows [32b, 32b+32) = core_b_out.T; output = [x | c_full].
"""

import numpy as np
import ml_dtypes

B, F, O, K = 256, 1024, 128, 8
NCORES = 8
IB = B // NCORES  # c rows owned per core

_cache = {}


def _build():
    from contextlib import ExitStack
    import concourse.bacc as bacc
    import concourse.tile as tile
    import concourse.mybir as mybir

    dt = mybir.dt
    Alu = mybir.AluOpType
    Act = mybir.ActivationFunctionType

    nc = bacc.Bacc(
        "TRN2", target_bir_lowering=False, debug=False, enable_asserts=False
    )
    w = nc.dram_tensor("w", (128, F // 128 * O * K), dt.float8e4, kind="ExternalInput").ap()
    xt = nc.dram_tensor("xt", (128, F // 128 * B), dt.float8e4, kind="ExternalInput").ap()
    mask = nc.dram_tensor("mask", (128, 64), dt.bfloat16, kind="ExternalInput").ap()
    dmask = nc.dram_tensor("dmask", (128, 128), dt.bfloat16, kind="ExternalInput").ap()
    eout = nc.dram_tensor("eb", (O, IB * 160), dt.bfloat16, kind="ExternalOutput").ap()

    FC = F // 128  # contraction chunks
    G = 8  # column groups (2 o-halves x 4 k-quarters)

    with ExitStack() as ctx:
        tc = ctx.enter_context(tile.TileContext(nc))
        inpool = ctx.enter_context(tc.tile_pool(name="inp", bufs=1))
        mpool = ctx.enter_context(tc.tile_pool(name="mt", bufs=1))
        dpool = ctx.enter_context(tc.tile_pool(name="d", bufs=16))
        cpool = ctx.enter_context(tc.tile_pool(name="c", bufs=1))

        # single DMA per input: dma_start issue costs ~0.6us each on the
        # sequencer, so chunked loads gate the GEMM on issue rate, not BW
        # w/xt are shipped pre-interleaved as the exact SBUF image, so these
        # DMAs are fully contiguous (strided loads run at ~half DMA BW)
        w_sb = inpool.tile([128, FC * O * K], dt.float8e4, tag="wsb")
        nc.sync.dma_start(w_sb[:], w)
        x_sb = inpool.tile([128, FC * B], dt.float8e4, tag="xsb")
        nc.sync.dma_start(x_sb[:], xt)
        mask_sb = inpool.tile([128, 64], dt.bfloat16, tag="mask")
        nc.sync.dma_start(mask_sb[:], mask)
        dmask_sb = inpool.tile([128, 128], dt.bfloat16, tag="dmask")
        nc.sync.dma_start(dmask_sb[:], dmask)
        x_tiles = [x_sb[:, fc * B : (fc + 1) * B] for fc in range(FC)]
        w_tiles = [x_sb and w_sb[:, fc * O * K : (fc + 1) * O * K] for fc in range(FC)]

        # Mt: partition p=(o%64)*2+k%2, free j; one tile per group g = 4*(o//64)+k//2
        mt = [
            mpool.tile([128, B], dt.bfloat16, tag=f"mt{g}", name=f"mt{g}")
            for g in range(G)
        ]
        # f32 copies of the first IB columns of each group (tensor_scalar scalars)
        mtf = [
            mpool.tile([128, IB], dt.float32, tag=f"mtf{g}", name=f"mtf{g}")
            for g in range(G)
        ]
        # negated f32 columns (activation-Abs bias for the ScalarE row variant)
        nmtf = [
            mpool.tile([128, IB], dt.float32, tag=f"nmtf{g}", name=f"nmtf{g}")
            for g in range(G)
        ]

        with tc.tile_pool(name="gps", bufs=4, space="PSUM") as gps:
            # host pre-permuted w: col = (g*64 + o%64)*2 + k%2, g=4*(o//64)+k//2
            # g-outer so mt[0] (and the pairwise loop) starts as soon as the
            # last w/x chunk lands rather than after the whole GEMM
            for g in range(G):
                gm = gps.tile([128, B], dt.float32, tag="gm", name=f"gm{g}")
                for fc in range(FC):
                    nc.tensor.matmul(
                        gm[:],
                        w_tiles[fc][:, g * 128 : (g + 1) * 128],
                        x_tiles[fc][:],
                        start=(fc == 0),
                        stop=(fc == FC - 1),
                    )
                nc.scalar.copy(mt[g][:], gm[:])
                nc.scalar.copy(mtf[g][:], mt[g][:, 0:IB])
                nc.scalar.mul(nmtf[g][:], mt[g][:, 0:IB], -1.0)

        # S[o, j] = sum_k M[j, o, k].  Two forms are kept:
        #  - sneg[q]: -S/4 in the duplicated (o%64)*2+k%2 partition layout
        #    (bf16): streamed as the first pairwise matmul of each row with
        #    start=True, so the mask-sum over partition pairs seeds the PSUM
        #    with -S_j/2.
        #  - neg_s_bias: -bf16(S) as f32 in o-layout: the per-partition exp
        #    bias.  bf16(S/4)*4 == bf16(S) exactly, so the diagonal argument
        #    cancels to exactly 0 for every row and e[:, i] is one constant
        #    column, extracted once (c_diag0).
        sneg = [
            mpool.tile([128, B], dt.bfloat16, tag=f"sneg{q}", name=f"sneg{q}")
            for q in range(2)
        ]
        s_bf = mpool.tile([128, IB], dt.bfloat16, tag="sbf")
        neg_s_bias = mpool.tile([128, IB], dt.float32, tag="negsb")
        with tc.tile_pool(name="sps", bufs=1, space="PSUM") as spsp:
            # sneg first: it gates the first DVE-row mask-matmuls of the loop;
            # its copies run on VectorE so they don't queue behind ScalarE work
            sd_ps = spsp.tile([128, B], dt.float32, tag="sdps")
            for q in range(2):
                for kh in range(4):
                    g = 4 * q + kh
                    nc.tensor.matmul(
                        sd_ps[:],
                        dmask_sb[:],
                        mt[g][:],
                        start=(kh == 0),
                        stop=(kh == 3),
                    )
                nc.vector.tensor_scalar(
                    sneg[q][:], sd_ps[:], -0.25, None, Alu.mult
                )
            s_ps = spsp.tile([128, B], dt.float32, tag="sps")
            for q in range(2):
                for kh in range(4):
                    g = 4 * q + kh
                    nc.tensor.matmul(
                        s_ps[64 * q : 64 * (q + 1), :],
                        mask_sb[:],
                        mt[g][:],
                        start=(kh == 0),
                        stop=(kh == 3),
                    )
            nc.scalar.copy(s_bf[:], s_ps[:, 0:IB])
            nc.scalar.mul(neg_s_bias[:], s_bf[:], -1.0)

        # shifted band store for the symmetric column-part: row i's exp window
        # (cols [i+1, i+129)) lands at slot (i%8)*160 + col inside band i//8;
        # gaps stay zero, bands are DMA'd out and reduced on the host
        band = [
            cpool.tile([128, 8 * 160], dt.bfloat16, tag=f"band{t}", name=f"band{t}")
            for t in range(4)
        ]
        for t in range(4):
            nc.scalar.memzero(band[t][:])

        with tc.tile_pool(name="dps", bufs=6, space="PSUM") as dps, tc.tile_pool(
            name="eps", bufs=2, space="PSUM"
        ) as eps:
            WIN = 128  # circular half-window: row i covers j in (i, i+128]
            first_mm = True
            for i in range(IB):
                # rows computed via ScalarE Abs instead of VectorE (balance);
                # two lead rows need no sneg so PE starts before the S chain
                # resolves, and none sit at the tail
                act_row = i in (0, 1, 4, 8, 12, 16, 20, 24)
                lo, hi = i + 1, i + 1 + WIN
                t, sl = divmod(i, 8)
                d = dpool.tile([128, G * WIN], dt.bfloat16, tag="d")
                for g in range(G):
                    if act_row:
                        nc.scalar.activation(
                            d[:, g * WIN : (g + 1) * WIN],
                            mt[g][:, lo:hi],
                            Act.Abs,
                            bias=nmtf[g][:, i : i + 1],
                        )
                    else:
                        nc.vector.tensor_scalar(
                            d[:, g * WIN : (g + 1) * WIN],
                            mt[g][:, lo:hi],
                            mtf[g][:, i : i + 1],
                            0.0,
                            Alu.subtract,
                            Alu.max,
                        )
                diffs = dps.tile([128, WIN], dt.float32, tag="diffs")
                for q in range(2):
                    mms = []
                    if not act_row:
                        # seed the PSUM quadrant with -S_j/2
                        mms.append(
                            nc.tensor.matmul(
                                diffs[64 * q : 64 * (q + 1), :],
                                mask_sb[:],
                                sneg[q][:, lo:hi],
                                start=True,
                                stop=False,
                                skip_group_check=True,
                            )
                        )
                    for kh in range(4):
                        g = 4 * q + kh
                        mms.append(
                            nc.tensor.matmul(
                                diffs[64 * q : 64 * (q + 1), :],
                                mask_sb[:],
                                d[:, g * WIN : (g + 1) * WIN],
                                start=(act_row and kh == 0),
                                stop=(kh == 3),
                                skip_group_check=True,
                            )
                        )
                    for mm in mms:
                        # every pairwise matmul reuses the identical mask
                        # weights: skip the LDWEIGHTS reload after the first
                        if not first_mm:
                            mm.ins.ldweights = False
                        first_mm = False
                # no accum_out: the host sums the band rows (row-part) and
                # the shifted columns (column-part) from the same eb output
                nc.scalar.activation(
                    band[t][:, sl * 160 + lo : sl * 160 + hi],
                    diffs[:],
                    Act.Exp,
                    scale=(-1.0 if act_row else -2.0),
                    bias=(0.0 if act_row else neg_s_bias[:, i : i + 1]),
                )
                if sl == 7:
                    # ship the finished band while the loop continues
                    nc.sync.dma_start(
                        eout[:, t * 1280 : (t + 1) * 1280], band[t][:]
                    )


    nc.compile()
    return nc


def _prep_inputs(x, T):
    bf16 = ml_dtypes.bfloat16
    # permute W columns to (q, kh, o%64, k%2) so every per-group lhsT slice of
    # the GEMM is contiguous (walrus: weights AP must have one free dim)
    Wp = (
        np.asarray(T, np.float32)
        .reshape(F, 2, 64, 4, 2)
        .transpose(0, 1, 3, 2, 4)
        .reshape(F, O * K)
    )
    fp8 = ml_dtypes.float8_e4m3
    # SBUF-image interleave: row p holds [chunk0 | chunk1 | ...] where
    # chunk c covers input features [128c, 128c+128)
    W = np.ascontiguousarray(
        Wp.reshape(F // 128, 128, O * K).transpose(1, 0, 2).reshape(128, -1)
    ).astype(fp8)
    xTf = x.T.astype(np.float32)
    mask = np.zeros((128, 64), dtype=bf16)
    mask[np.arange(128), np.arange(128) // 2] = 1.0
    dmask = np.zeros((128, 128), dtype=bf16)
    p = np.arange(128)
    dmask[:, :] = (p[:, None] // 2 == p[None, :] // 2).astype(bf16)
    in_maps = []
    for b in range(NCORES):
        in_maps.append(
            {
                "w": W,
                "mask": mask,
                "dmask": dmask,
                "xt": np.ascontiguousarray(
                    np.roll(xTf, -IB * b, axis=1)
                    .reshape(F // 128, 128, B)
                    .transpose(1, 0, 2)
                    .reshape(128, -1)
                ).astype(fp8),
            }
        )
    return in_maps


def _assemble(x, results):
    # row-part from each core plus banded column-part contributions from the
    # core itself and its four predecessors (window j in (i, i+128] spans up
    # to 5 row-blocks; block wrap follows the per-core batch rotation)
    c2s, rowp = [], []
    for r_ in results:
        eb = r_["eb"].astype(np.float32).reshape(O, IB, 160)
        c2s.append(eb.sum(axis=1))  # (O, 160) column-part bands
        rowp.append(eb.sum(axis=2))  # (O, IB) row-part sums
    c_full = np.zeros((B, O), np.float32)
    r = np.arange(IB)
    for b in range(NCORES):
        rows = IB * b + r
        c_full[rows] = rowp[b].T
        for t in range(5):
            c_full[rows] += c2s[(b - t) % NCORES][:, 32 * t + r].T
    return np.concatenate([np.asarray(x, np.float32), c_full], axis=1)


def _get_nc():
    if "nc" not in _cache:
        _cache["nc"] = _build()
    return _cache["nc"]


def kernel(x, T):
    from concourse.bass_utils import run_bass_kernel_spmd

    x = np.asarray(x)
    T = np.asarray(T)
    nc = _get_nc()
    res = run_bass_kernel_spmd(nc, _prep_inputs(x, T), list(range(NCORES)))
    return _assemble(x, res.results)


def run_traced(x, T, **kwargs):
    """Like kernel() but returns (output, BassKernelResults) with tracing on."""
    from concourse.bass_utils import run_bass_kernel_spmd

    x = np.asarray(x)
    T = np.asarray(T)
    nc = _get_nc()
    res = run_bass_kernel_spmd(
        nc, _prep_inputs(x, T), list(range(NCORES)), trace=True, **kwargs
    )
    return _assemble(x, res.results), res



# revision 2
# speedup vs baseline: 2.0080x; 2.0080x over previous
"""Trainium2 Bass kernel for nn_MinibatchDiscrimination (B=256, F=1024, O=128, K=8).

out = concat([x, c]),  c[i,o] = sum_{j!=i} exp(-sum_k |M[j,o,k]-M[i,o,k]|),
M = x @ T.

Identity: |a-b| = 2*max(a,b) - a - b, so with S = sum_k M and H = sum_k max:
  exp(-diffs) = exp(-2*H + S_i + S_j).

Layout: partition p = o (all 128 output features), k is the group dim.
  mt [128, (k 8, j 160)] bf16 from an fp8 DoubleRow GEMM (k-major W image).
  tt-max per k: d[k][o, delta*32+i] = max(mt[o, k*160+1+delta+i], mt[o, k*160+i])
    - one batched DVE tensor_tensor(max) per k over all 32 rows x 128 window,
      (delta,i) APs with i innermost (unit stride) so DVE 2x mode engages.
  ksum: PSUM banks tile the DELTA axis (bank b = delta in [16b,16b+16) x all i),
    so every matmul rhs is a flat contiguous 512-col slice of d[k]; the
    "mask" is a 128x128 identity so matmuls are full-width PSUM accumulates.
  S: identity-matmul accumulation over the 8 k-blocks of mt; sneg = -S/2
    (ScalarE); ssum[o, delta*32+i] = -(S_i + S_j)/2 via one batched DVE add
    with the same (delta,i) window APs; one seed matmul per bank closes the
    accumulation group.
  exp per bank: flat [128,512] PSUM -> band slice (band col = delta*32+i),
    scale=-2, bias=0, fully contiguous; band DMA'd out in 4 chunks.

Distribution: c rows sharded across 8 cores (32 each) via host-side column
rotation of x^T; every core runs the full GEMM redundantly (no collectives).
Host assembles row-part + shifted column-part and concats with x.
"""

import numpy as np
import ml_dtypes

B, F, O, K = 256, 1024, 128, 8
NCORES = 8
IB = B // NCORES  # 32 rows per core
WIN = 128
JW = 160  # local j extent
FC = F // 128

_cache = {}


def _build():
    from contextlib import ExitStack
    import concourse.bacc as bacc
    import concourse.tile as tile
    import concourse.mybir as mybir
    from concourse.bass import AP

    dt = mybir.dt
    Alu = mybir.AluOpType
    Act = mybir.ActivationFunctionType
    DR = mybir.MatmulPerfMode.DoubleRow

    nc = bacc.Bacc(
        "TRN2", target_bir_lowering=False, debug=False, enable_asserts=False
    )
    w0 = nc.dram_tensor("w0", (128, 4 * FC * 128), dt.float8e4, kind="ExternalInput").ap()
    w1 = nc.dram_tensor("w1", (128, 4 * FC * 128), dt.float8e4, kind="ExternalInput").ap()
    xtd = nc.dram_tensor("xt", (128, FC * JW), dt.float8e4, kind="ExternalInput").ap()
    idd = nc.dram_tensor("idm", (128, 128), dt.bfloat16, kind="ExternalInput").ap()
    eout = nc.dram_tensor("eb", (O, WIN * IB), dt.bfloat16, kind="ExternalOutput").ap()

    NB = 8  # delta-block PSUM banks
    NBA = 6  # banks coexisting with the GEMM pool

    with ExitStack() as ctx:
        tc = ctx.enter_context(tile.TileContext(nc))
        inpool = ctx.enter_context(tc.tile_pool(name="inp", bufs=1))
        mpool = ctx.enter_context(tc.tile_pool(name="mt", bufs=1))
        dpool = ctx.enter_context(tc.tile_pool(name="d", bufs=1))
        cpool = ctx.enter_context(tc.tile_pool(name="c", bufs=1))

        w_sb = inpool.tile([128, 8 * FC * 128], dt.float8e4, tag="wsb")
        x_sb = inpool.tile([128, FC * JW], dt.float8e4, tag="xsb")
        id_sb = inpool.tile([128, 128], dt.bfloat16, tag="idm")
        nc.sync.dma_start(w_sb[:, 0 : 4 * FC * 128], w0)
        nc.gpsimd.dma_start(w_sb[:, 4 * FC * 128 :], w1)
        nc.scalar.dma_start(x_sb[:], xtd)
        nc.scalar.dma_start(id_sb[:], idd)

        mt = mpool.tile([128, 8 * JW], dt.bfloat16, tag="mt")
        sneg = mpool.tile([128, JW], dt.bfloat16, tag="sneg")
        ssum = mpool.tile([128, WIN * IB], dt.bfloat16, tag="ssum")
        d = [
            dpool.tile([128, WIN * IB], dt.bfloat16, tag=f"d{k}", name=f"d{k}")
            for k in range(K)
        ]
        band = cpool.tile([128, WIN * IB], dt.bfloat16, tag="band")

        def win_ap(tile_ap, base_off, sd, si):
            """[p, (delta: 128 x stride sd, i: 32 x stride si)], i innermost."""
            prow = list(tile_ap.ap[0])
            return AP(tile_ap.tensor, base_off, [prow, [sd, WIN], [si, IB]])

        with tc.tile_pool(name="kpsA", bufs=NBA, space="PSUM") as kpsA:
            pts = {}
            for b in range(NBA):
                pts[b] = kpsA.tile([128, 512], dt.float32, tag="pt", name=f"pt{b}")

            def layer(k, banks):
                for b in banks:
                    nc.tensor.matmul(
                        pts[b][:],
                        id_sb[:],
                        d[k][:, b * 512 : (b + 1) * 512],
                        start=(k == 0),
                        stop=False,
                        skip_group_check=True,
                    )

            with tc.tile_pool(name="gps", bufs=2, space="PSUM") as gps:
                for k in range(K):
                    gm = gps.tile([128, JW], dt.float32, tag="gm", name=f"gm{k}")
                    for pr in range(FC // 2):
                        base = k * FC * 128 + pr * 256
                        nc.tensor.matmul(
                            gm[:],
                            w_sb[:, base : base + 256].rearrange(
                                "p (two m) -> p two m", two=2
                            ),
                            x_sb[:, pr * 2 * JW : (pr + 1) * 2 * JW].rearrange(
                                "p (two n) -> p two n", two=2
                            ),
                            start=(pr == 0),
                            stop=(pr == FC // 2 - 1),
                            perf_mode=DR,
                        )
                    nc.scalar.copy(mt[:, k * JW : (k + 1) * JW], gm[:])
                    nc.vector.tensor_tensor(
                        d[k][:].rearrange("p (dd i) -> p dd i", dd=WIN),
                        win_ap(mt[:], k * JW + 1, 1, 1),
                        win_ap(mt[:], k * JW, 0, 1),
                        Alu.max,
                    )
                    if 2 <= k:
                        layer(k - 2, range(NBA))

                # S = sum_k M via identity-matmul accumulation over k blocks
                sp = gps.tile([128, JW], dt.float32, tag="gm", name="sp")
                for k in range(K):
                    nc.tensor.matmul(
                        sp[:],
                        id_sb[:],
                        mt[:, k * JW : (k + 1) * JW],
                        start=(k == 0),
                        stop=(k == K - 1),
                    )
                nc.scalar.mul(sneg[:], sp[:], -0.5)
                nc.vector.tensor_tensor(
                    ssum[:].rearrange("p (dd i) -> p dd i", dd=WIN),
                    win_ap(sneg[:], 1, 1, 1),
                    win_ap(sneg[:], 0, 0, 1),
                    Alu.add,
                )
                layer(K - 2, range(NBA))
                layer(K - 1, range(NBA))

            with tc.tile_pool(name="kpsB", bufs=NB - NBA, space="PSUM") as kpsB:
                for b in range(NBA, NB):
                    pts[b] = kpsB.tile([128, 512], dt.float32, tag="pt", name=f"pt{b}")
                for k in range(K):
                    layer(k, range(NBA, NB))
                for b in range(NB):
                    # seed closes the accumulation group: h = H - (S_i+S_j)/2
                    nc.tensor.matmul(
                        pts[b][:],
                        id_sb[:],
                        ssum[:, b * 512 : (b + 1) * 512],
                        start=False,
                        stop=True,
                        skip_group_check=True,
                    )
                    nc.scalar.activation(
                        band[:, b * 512 : (b + 1) * 512],
                        pts[b][:],
                        Act.Exp,
                        scale=-2.0,
                    )
                    if b % 2 == 1:
                        nc.gpsimd.dma_start(
                            eout[:, (b - 1) * 512 : (b + 1) * 512],
                            band[:, (b - 1) * 512 : (b + 1) * 512],
                        )

    nc.compile()
    return nc


def _prep_inputs(x, T):
    bf16 = ml_dtypes.bfloat16
    fp8 = ml_dtypes.float8_e4m3
    # W image: chunk k (k-major), col o; row p = f%128, col = k*FC*128 + fc*128 + o
    Wp = np.asarray(T, np.float32).transpose(2, 1, 0)  # (K, O, F)
    Wimg = (
        Wp.reshape(K, O, FC, 128).transpose(3, 0, 2, 1).reshape(128, -1)
    )  # (p, k*FC*O)
    Wimg = np.ascontiguousarray(Wimg).astype(fp8)
    xTf = np.asarray(x, np.float32).T  # (F, B)
    idm = np.eye(128, dtype=bf16)
    in_maps = []
    for b in range(NCORES):
        xl = np.roll(xTf, -IB * b, axis=1)[:, :JW]  # (F, 160)
        xi = np.ascontiguousarray(
            xl.reshape(FC, 128, JW).transpose(1, 0, 2).reshape(128, -1)
        ).astype(fp8)
        in_maps.append(
            {
                "w0": Wimg[:, : 4 * FC * 128],
                "w1": Wimg[:, 4 * FC * 128 :],
                "xt": xi,
                "idm": idm,
            }
        )
    return in_maps


def _assemble(x, results):
    c = np.zeros((B, O), np.float32)
    ar = np.arange(IB)
    for b in range(NCORES):
        E = results[b]["eb"].astype(np.float32).reshape(O, WIN, IB)  # (o, delta, i)
        rows = (IB * b + ar) % B
        c[rows] += E.sum(axis=1).T  # row part: sum over delta
        colsum = np.zeros((O, IB + WIN), np.float32)  # local j in [0, 160)
        for i in range(IB):
            colsum[:, i + 1 : i + 1 + WIN] += E[:, :, i]
        gj = (IB * b + np.arange(IB + WIN)) % B
        np.add.at(c, gj, colsum.T)
    return np.concatenate([np.asarray(x, np.float32), c], axis=1)


def _get_nc():
    if "nc" not in _cache:
        _cache["nc"] = _build()
    return _cache["nc"]


def kernel(x, T):
    from concourse.bass_utils import run_bass_kernel_spmd

    x = np.asarray(x)
    T = np.asarray(T)
    nc = _get_nc()
    res = run_bass_kernel_spmd(nc, _prep_inputs(x, T), list(range(NCORES)))
    return _assemble(x, res.results)


def run_traced(x, T, **kwargs):
    from concourse.bass_utils import run_bass_kernel_spmd

    x = np.asarray(x)
    T = np.asarray(T)
    nc = _get_nc()
    res = run_bass_kernel_spmd(
        nc, _prep_inputs(x, T), list(range(NCORES)), trace=True, **kwargs
    )
    return _assemble(x, res.results), res
